# revision 1
# baseline (speedup 1.0000x reference)
"""Performer multi-head linear-attention block on 8 TRN2 NeuronCores.

Sharding: node dim N split 8 ways (2048 nodes/batch/core). Weights replicated.
Per-core partial kp_sum + context are AllReduced ([65, 16*266] fp32 ~1.1MB),
then each core finishes its own output shard.

Layout plan (per core, ROWS = B*2048 = 4096):
  hT   [c(4x128), n4096]   <- PE-transpose of h shard
  kT/qT[c_out(4x128), n]   <- lhsT=W (natural), rhs=hT      (transposed QKV)
  v    [n, c_out]          <- lhsT=hT chunk,    rhs=Wv      (natural V) + ones col
  kp   [n, m384]           <- lhsT=kT head slice, rhs=projT (natural, m-padded)
  ctxT [65, m]   = v_aug.T @ kp  (contract n), accumulated in SBUF, AllReduce,
                   then PE-transposed to ctx [m(3x128), 65] per (b,head)
  qpT  [m, n]              <- lhsT=projT chunk, rhs=qT head slice
  outT [65, n512] = ctx_aug.T @ qpT ; row 64 = qp.kp_sum -> reciprocal -> d_inv
  OUTT [c, n]  = outT[0:64] * d_inv  (broadcast via ones-matmul)
  y    [n,512] <- lhsT=OUTT chunk, rhs=Wo; + bo; LayerNorm; DMA out
"""

import numpy as np

import concourse.bass as bass
import concourse.bacc as bacc
import concourse.tile as tile
from concourse import mybir
from concourse.masks import make_identity

F32 = mybir.dt.float32
F32R = mybir.dt.float32r

B = 2
N = 16384
C = 512
H = 8
D = 64          # C // H
M = 266         # int(D * log(D))
MP = 384        # M padded to 3*128
MC = 3          # m chunks
KEPS = 1e-3
LNEPS = 1e-5
NCORES = 8
NSH = N // NCORES          # 2048 nodes per batch per core
ROWS = B * NSH             # 4096 rows per core
NB = ROWS // 512           # 8 chunks of 512 rows
P = 128


def r(ap):
    return ap.bitcast(F32R)


def build_nc():
    nc = bacc.Bacc("TRN2", target_bir_lowering=False, debug=False,
                   num_devices=NCORES)

    h_h = nc.dram_tensor("h", [ROWS, C], F32, kind="ExternalInput")
    Wq_h = nc.dram_tensor("Wq", [C, C], F32, kind="ExternalInput")
    Wk_h = nc.dram_tensor("Wk", [C, C], F32, kind="ExternalInput")
    Wv_h = nc.dram_tensor("Wv", [C, C], F32, kind="ExternalInput")
    Wo_h = nc.dram_tensor("Wo", [C, C], F32, kind="ExternalInput")
    bq_h = nc.dram_tensor("bq", [C], F32, kind="ExternalInput")
    bk_h = nc.dram_tensor("bk", [C], F32, kind="ExternalInput")
    bv_h = nc.dram_tensor("bv", [C], F32, kind="ExternalInput")
    bo_h = nc.dram_tensor("bo", [C], F32, kind="ExternalInput")
    proj_h = nc.dram_tensor("proj", [M, D], F32, kind="ExternalInput")
    lng_h = nc.dram_tensor("ln_g", [C], F32, kind="ExternalInput")
    lnb_h = nc.dram_tensor("ln_b", [C], F32, kind="ExternalInput")
    out_h = nc.dram_tensor("out", [ROWS, C], F32, kind="ExternalOutput")

    h = h_h.ap()
    out = out_h.ap()

    def bcast_row(hnd):
        ap = hnd.ap()
        return bass.AP(tensor=ap.tensor, offset=ap.offset,
                       ap=[[0, P]] + list(ap.ap))

    with tile.TileContext(nc) as tc:
        import contextlib
        with contextlib.ExitStack() as ctx:
            consts = ctx.enter_context(tc.tile_pool(name="consts", bufs=1))
            persist = ctx.enter_context(tc.tile_pool(name="persist", bufs=1))

            identity = consts.tile([P, P], F32)
            make_identity(nc, identity)
            ones_1x64 = consts.tile([1, D], F32)
            nc.vector.memset(ones_1x64, 1.0)
            lneps_sb = consts.tile([P, 1], F32)
            nc.vector.memset(lneps_sb, LNEPS)

            # replicated weights
            Wq_sb = consts.tile([P, 4, C], F32)
            for ci in range(4):
                nc.sync.dma_start(r(Wq_sb[:, ci, :]), r(Wq_h.ap()[ci * P:(ci + 1) * P, :]))
            # Wo in 8 chunks of 64 rows, all at base partition 0, so the final
            # matmul can consume OUTT head blocks without partition shifts
            Wo_sb = consts.tile([D, H, C], F32)
            for hd in range(H):
                nc.sync.dma_start(r(Wo_sb[:, hd, :]), r(Wo_h.ap()[hd * D:(hd + 1) * D, :]))

            # per-partition bias columns for transposed layouts
            bq_sb = consts.tile([P, 4], F32)
            bk_sb = consts.tile([P, 4], F32)
            for ci in range(4):
                nc.sync.dma_start(bq_sb[:, ci:ci + 1], bq_h.ap()[ci * P:(ci + 1) * P])
                nc.sync.dma_start(bk_sb[:, ci:ci + 1], bk_h.ap()[ci * P:(ci + 1) * P])

            # free-dim broadcast rows
            bv_b = consts.tile([P, C], F32)
            bo_b = consts.tile([P, C], F32)
            lng_b = consts.tile([P, C], F32)
            lnb_b = consts.tile([P, C], F32)
            nc.gpsimd.dma_start(out=bv_b, in_=bcast_row(bv_h))
            nc.gpsimd.dma_start(out=bo_b, in_=bcast_row(bo_h))
            nc.gpsimd.dma_start(out=lng_b, in_=bcast_row(lng_h))
            nc.gpsimd.dma_start(out=lnb_b, in_=bcast_row(lnb_h))

            # proj -> projT [64, 384] zero-padded, duplicated to partitions
            # 64:128 so odd heads (base partition 64) have a matching operand
            proj_sb = consts.tile([P, MC, D], F32)
            nc.vector.memset(proj_sb, 0.0)
            nc.sync.dma_start(proj_sb[:, 0, :], proj_h.ap()[0:128, :])
            nc.sync.dma_start(proj_sb[:, 1, :], proj_h.ap()[128:256, :])
            nc.sync.dma_start(proj_sb[0:10, 2, :], proj_h.ap()[256:266, :])
            projT = consts.tile([P, MP], F32)
            with tc.tile_pool(name="pt_proj", bufs=1, space="PSUM") as ptp:
                for mc in range(MC):
                    pt = ptp.tile([D, P], F32, tag="ptproj")
                    nc.tensor.transpose(pt, proj_sb[:, mc, :], identity)
                    nc.vector.tensor_copy(r(projT[0:D, mc * P:(mc + 1) * P]), pt)
            nc.sync.dma_start(r(projT[D:P, :]), r(projT[0:D, :]))

            # persistent state
            hT = persist.tile([P, 4, ROWS], F32)                # 64KB/part
            ctx_sb = persist.tile([P, B * H, MC, 65], F32)      # ctx_aug [m,65]
            nc.vector.memset(ctx_sb, 0.0)

            acc_pool = tc.alloc_tile_pool(name="acc", bufs=1)
            ctxT_acc = acc_pool.tile([65, B * H, M], F32)

            # ---------------- pass 1: k, v -> kp -> context partials --------
            with contextlib.ExitStack() as p1:
                w1 = p1.enter_context(tc.tile_pool(name="w1", bufs=1))
                Wk_sb = w1.tile([P, 4, C], F32)
                Wv_sb = w1.tile([P, 4, C], F32)
                for ci in range(4):
                    nc.sync.dma_start(r(Wk_sb[:, ci, :]), r(Wk_h.ap()[ci * P:(ci + 1) * P, :]))
                    nc.sync.dma_start(r(Wv_sb[:, ci, :]), r(Wv_h.ap()[ci * P:(ci + 1) * P, :]))

                stage = p1.enter_context(tc.tile_pool(name="stage", bufs=2))
                kt_pool = p1.enter_context(tc.tile_pool(name="ktp", bufs=2))
                v_pool = p1.enter_context(tc.tile_pool(name="vp", bufs=2))
                kp_pool = p1.enter_context(tc.tile_pool(name="kpp", bufs=2))
                ps_t = p1.enter_context(tc.tile_pool(name="ps_t", bufs=2, space="PSUM"))
                ps_qkv = p1.enter_context(tc.tile_pool(name="ps_qkv", bufs=2, space="PSUM"))
                ps_kp = p1.enter_context(tc.tile_pool(name="ps_kp", bufs=2, space="PSUM"))
                ps_ctx = p1.enter_context(tc.tile_pool(name="ps_ctx", bufs=2, space="PSUM"))

                for nb in range(NB):
                    b = nb // 4
                    n0 = nb * 512
                    # transpose h chunk into persistent hT
                    for ns in range(4):
                        h_stage = stage.tile([P, C], F32, tag="h_stage")
                        nc.sync.dma_start(h_stage, h[n0 + ns * P:n0 + (ns + 1) * P, :])
                        for ci in range(4):
                            pt = ps_t.tile([P, P], F32, tag="pt")
                            nc.tensor.transpose(pt, h_stage[:, ci * P:(ci + 1) * P], identity)
                            nc.scalar.copy(r(hT[:, ci, n0 + ns * P:n0 + (ns + 1) * P]), pt)

                    # kT chunk [co 4][128, 512]
                    kT = kt_pool.tile([P, 4, 512], F32, tag="kT")
                    for co in range(4):
                        pk = ps_qkv.tile([P, 512], F32, tag="pqkv")
                        for ci in range(4):
                            nc.tensor.matmul(pk, r(Wk_sb[:, ci, co * P:(co + 1) * P]),
                                             r(hT[:, ci, n0:n0 + 512]),
                                             start=(ci == 0), stop=(ci == 3))
                        nc.vector.tensor_scalar(r(kT[:, co, :]), pk, bk_sb[:, co:co + 1],
                                                None, mybir.AluOpType.add)

                    # v chunk natural, augmented with ones col per head
                    vaug = v_pool.tile([P, 4, H, 65], F32, tag="vaug")
                    nc.vector.memset(vaug[:, :, :, 64:65], 1.0)
                    for ns in range(4):
                        pv = ps_qkv.tile([P, 512], F32, tag="pqkv")
                        for ci in range(4):
                            nc.tensor.matmul(pv, r(hT[:, ci, n0 + ns * P:n0 + (ns + 1) * P]),
                                             r(Wv_sb[:, ci, :]),
                                             start=(ci == 0), stop=(ci == 3))
                        for hd in range(H):
                            nc.vector.tensor_tensor(r(vaug[:, ns, hd, 0:64]),
                                                    pv[:, hd * D:(hd + 1) * D],
                                                    bv_b[:, hd * D:(hd + 1) * D],
                                                    mybir.AluOpType.add)

                    for hd in range(H):
                        pair = b * H + hd
                        po = (hd % 2) * D
                        co = hd // 2
                        kp = kp_pool.tile([P, 4, M], F32, tag="kp")
                        for ns in range(4):
                            pkp = ps_kp.tile([P, M], F32, tag="pkp")
                            nc.tensor.matmul(pkp,
                                             r(kT[po:po + D, co, ns * P:(ns + 1) * P]),
                                             r(projT[po:po + D, 0:M]),
                                             start=True, stop=True)
                            nc.vector.tensor_scalar(r(kp[:, ns, :]), pkp,
                                                    0.0, KEPS,
                                                    mybir.AluOpType.max,
                                                    mybir.AluOpType.add)
                        pctx = ps_ctx.tile([65, M], F32, tag="pctx")
                        for ns in range(4):
                            nc.tensor.matmul(pctx, r(vaug[:, ns, hd, :]),
                                             r(kp[:, ns, :]),
                                             start=(ns == 0), stop=(ns == 3))
                        if nb % 4 == 0:
                            nc.vector.tensor_copy(ctxT_acc[:, pair, :], pctx)
                        else:
                            nc.vector.tensor_tensor(ctxT_acc[:, pair, :],
                                                    ctxT_acc[:, pair, :],
                                                    pctx,
                                                    mybir.AluOpType.add)

            # ---------------- AllReduce of [65, 16, 266] ---------------------
            with tc.tile_pool(name="dram", bufs=1, space="DRAM") as dram:
                cc_in = dram.tile([65, B * H, M], F32)
                cc_out = dram.tile([65, B * H, M], F32, addr_space="Shared")
                nc.gpsimd.dma_start(cc_in[:], ctxT_acc[:])
                nc.gpsimd.collective_compute(
                    "AllReduce", mybir.AluOpType.add,
                    replica_groups=[list(range(NCORES))],
                    ins=[cc_in.opt()], outs=[cc_out.opt()],
                )
                nc.gpsimd.dma_start(ctxT_acc[:], cc_out[:])

            # transpose reduced ctxT [65, m] -> ctx_sb [m, 65] per pair
            with tc.tile_pool(name="ps_ctxt", bufs=2, space="PSUM") as ps_ctxt:
                for pair in range(B * H):
                    for mc in range(MC):
                        mlo = mc * P
                        mhi = min(M, (mc + 1) * P)
                        ptc = ps_ctxt.tile([P, 65], F32, tag="ptc")
                        nc.tensor.transpose(ptc[0:mhi - mlo, :],
                                            ctxT_acc[:, pair, mlo:mhi],
                                            identity[0:65, 0:65])
                        nc.scalar.copy(r(ctx_sb[0:mhi - mlo, pair, mc, :]),
                                       ptc[0:mhi - mlo, :])
            acc_pool.release()

            # ---------------- pass 2: q -> qp -> out -> Wo -> LN -------------
            with contextlib.ExitStack() as p2:
                qt_pool = p2.enter_context(tc.tile_pool(name="qtp", bufs=2))
                qp_pool = p2.enter_context(tc.tile_pool(name="qpp", bufs=2))
                outt_pool = p2.enter_context(tc.tile_pool(name="outtp", bufs=2))
                dinv_pool = p2.enter_context(tc.tile_pool(name="dinvp", bufs=2))
                y_pool = p2.enter_context(tc.tile_pool(name="yp", bufs=2))
                st_pool = p2.enter_context(tc.tile_pool(name="stp", bufs=2))
                ps_big = p2.enter_context(tc.tile_pool(name="ps_big", bufs=2, space="PSUM"))
                ps_out = p2.enter_context(tc.tile_pool(name="ps_out", bufs=2, space="PSUM"))
                ps_b = p2.enter_context(tc.tile_pool(name="ps_b", bufs=1, space="PSUM"))
                ps_y = p2.enter_context(tc.tile_pool(name="ps_y", bufs=2, space="PSUM"))

                for nb in range(NB):
                    b = nb // 4
                    n0 = nb * 512
                    qT = qt_pool.tile([P, 4, 512], F32, tag="qT")
                    for co in range(4):
                        pq = ps_big.tile([P, 512], F32, tag="pbig")
                        for ci in range(4):
                            nc.tensor.matmul(pq, r(Wq_sb[:, ci, co * P:(co + 1) * P]),
                                             r(hT[:, ci, n0:n0 + 512]),
                                             start=(ci == 0), stop=(ci == 3))
                        nc.vector.tensor_scalar(r(qT[:, co, :]), pq, bq_sb[:, co:co + 1],
                                                None, mybir.AluOpType.add)

                    OUTT = outt_pool.tile([D, H, 512], F32, tag="OUTT")
                    for hd in range(H):
                        pair = b * H + hd
                        po = (hd % 2) * D
                        co = hd // 2
                        qp = qp_pool.tile([P, MC, 512], F32, tag="qp")
                        for mc in range(MC):
                            pqp = ps_big.tile([P, 512], F32, tag="pbig")
                            nc.tensor.matmul(pqp,
                                             r(projT[po:po + D, mc * P:(mc + 1) * P]),
                                             r(qT[po:po + D, co, :]),
                                             start=True, stop=True)
                            # rows >= 266-mc*128 are padding; ctx_sb is zero
                            # there, so eps in padded rows contributes nothing
                            nc.vector.tensor_scalar(r(qp[:, mc, :]), pqp, 0.0, KEPS,
                                                    mybir.AluOpType.max,
                                                    mybir.AluOpType.add)
                        pout = ps_out.tile([65, 512], F32, tag="pout")
                        for mc in range(MC):
                            nc.tensor.matmul(pout, r(ctx_sb[:, pair, mc, :]),
                                             r(qp[:, mc, :]),
                                             start=(mc == 0), stop=(mc == 2))
                        dinv = dinv_pool.tile([1, 512], F32, tag="dinv")
                        nc.vector.reciprocal(dinv, pout[64:65, :])
                        pb = ps_b.tile([D, 512], F32, tag="pb")
                        nc.tensor.matmul(pb, ones_1x64, dinv, start=True, stop=True)
                        dinvb = dinv_pool.tile([D, 512], F32, tag="dinvb")
                        nc.scalar.copy(dinvb, pb)
                        nc.vector.tensor_tensor(r(OUTT[:, hd, :]), pout[0:D, :],
                                                dinvb, mybir.AluOpType.mult)

                    for ns in range(4):
                        py = ps_y.tile([P, 512], F32, tag="py")
                        for hd in range(H):
                            nc.tensor.matmul(py, r(OUTT[:, hd, ns * P:(ns + 1) * P]),
                                             r(Wo_sb[:, hd, :]),
                                             start=(hd == 0), stop=(hd == H - 1))
                        ytmp = y_pool.tile([P, C], F32, tag="ytmp")
                        nc.vector.tensor_tensor(ytmp, py, bo_b, mybir.AluOpType.add)
                        stats = st_pool.tile([P, 6], F32, tag="stats")
                        nc.vector.bn_stats(stats, ytmp)
                        mv = st_pool.tile([P, 2], F32, tag="mv")
                        nc.vector.bn_aggr(mv, stats)
                        std = st_pool.tile([P, 1], F32, tag="std")
                        nc.scalar.activation(std, mv[:, 1:2],
                                             mybir.ActivationFunctionType.Sqrt,
                                             bias=lneps_sb[:])
                        rstd = st_pool.tile([P, 1], F32, tag="rstd")
                        nc.vector.reciprocal(rstd, std)
                        ynorm = y_pool.tile([P, C], F32, tag="ynorm")
                        nc.vector.tensor_scalar(ynorm, ytmp, mv[:, 0:1], rstd,
                                                mybir.AluOpType.subtract,
                                                mybir.AluOpType.mult)
                        yfin = y_pool.tile([P, C], F32, tag="yfin")
                        nc.vector.tensor_tensor(yfin, ynorm, lng_b,
                                                mybir.AluOpType.mult)
                        nc.vector.tensor_tensor(yfin, yfin, lnb_b,
                                                mybir.AluOpType.add)
                        nc.sync.dma_start(out[n0 + ns * P:n0 + (ns + 1) * P, :], yfin)

    return nc


_NC = None
LAST_RESULT = None


def make_in_maps(inputs):
    h = np.ascontiguousarray(inputs["h"], dtype=np.float32)
    shared = {
        k: np.ascontiguousarray(inputs[k], dtype=np.float32)
        for k in ["Wq", "Wk", "Wv", "Wo", "bq", "bk", "bv", "bo",
                  "proj", "ln_g", "ln_b"]
    }
    in_maps = []
    for c in range(NCORES):
        shard = np.ascontiguousarray(
            h[:, c * NSH:(c + 1) * NSH, :].reshape(ROWS, C))
        m = {"h": shard}
        m.update(shared)
        in_maps.append(m)
    return in_maps


def get_nc():
    global _NC
    if _NC is None:
        _NC = build_nc()
        _NC.finalize()
    return _NC


def kernel(**inputs):
    global LAST_RESULT
    nc = get_nc()
    from concourse.bass_utils import run_bass_kernel_spmd

    in_maps = make_in_maps(inputs)
    res = run_bass_kernel_spmd(nc, in_maps, core_ids=list(range(NCORES)))
    LAST_RESULT = res
    out = np.empty((B, N, C), dtype=np.float32)
    for c in range(NCORES):
        out[:, c * NSH:(c + 1) * NSH, :] = res.results[c]["out"].reshape(B, NSH, C)
    return out



# revision 23
# speedup vs baseline: 104.7868x; 104.7868x over previous
"""Performer multi-head linear-attention block on 8 TRN2 NeuronCores.

Sharding: node dim N split 8 ways (2048 nodes/batch/core); weights
replicated. Per-core partial contexts are AllReduced (two collectives,
one per batch, so each overlaps compute), then each core finishes its
own output shard.

v2 design (vs v1 baseline):
- bf16 operands everywhere on the matmul path (PSUM accumulates fp32);
  host prepacks h/W/projT as bf16 so DMA-transpose (2-byte only) gives
  hT without any PE transposes.
- Stage A (per 512-row block): qT/kT/v projections straight from hT,
  dense N=512 matmuls only.
- Stage B (head-pair-major): kp -> context accumulated IN PSUM across
  all 16 row-chunks of a batch, in [m, 65] orientation, so no DVE
  accumulation and no post-AllReduce transposes. PSUM is DMAd straight
  to the collective input buffer.
- Stage C: two AllReduces ([266,8,65] fp32 each); AR(b0) overlaps
  stage B of b1, AR(b1) overlaps output pass of b0.
- Stage D (per 512-row block): qp -> out -> 1/den (packed per head
  pair, Reciprocal on the Scalar engine) -> Wo (head pairs give plain
  K=128 matmuls) -> LayerNorm (Rsqrt + per-partition affine on ACT).
"""

import numpy as np

import concourse.bass as bass
import concourse.bacc as bacc
import concourse.tile as tile
from concourse import mybir
from concourse.dve_ops import RECIP_APPROX_FAST_CONSTS, RECIPROCAL_APPROX_FAST

F32 = mybir.dt.float32
F32R = mybir.dt.float32r
BF16 = mybir.dt.bfloat16


def r(ap):
    return ap.bitcast(F32R)
AF = mybir.ActivationFunctionType
OP = mybir.AluOpType

B = 2
N = 16384
C = 512
H = 8
D = 64          # C // H
M = 266         # int(D * log(D))
MP = 384        # M padded to 3*128
MC = 3          # m chunks (128, 128, 10)
MCW = [128, 128, 10]
KEPS = 1e-3
LNEPS = 1e-5
NCORES = 8
NSH = N // NCORES          # 2048 nodes per batch per core
ROWS = B * NSH             # 4096 rows per core
NB = ROWS // 512           # 8 blocks of 512 rows
NCH = NSH // 128           # 16 chunks of 128 rows per batch
P = 128


def build_nc(ncores=NCORES, dbg=False):
    nc = bacc.Bacc("TRN2", target_bir_lowering=False, debug=False,
                   num_devices=ncores)

    h_h = nc.dram_tensor("h", [ROWS, C], BF16, kind="ExternalInput")
    Wq_h = nc.dram_tensor("Wq", [C, C], BF16, kind="ExternalInput")
    Wk_h = nc.dram_tensor("Wk", [C, C], BF16, kind="ExternalInput")
    Wv_h = nc.dram_tensor("Wv", [C, C], BF16, kind="ExternalInput")
    Wo_h = nc.dram_tensor("Wo", [C, C], BF16, kind="ExternalInput")
    bq_h = nc.dram_tensor("bq", [C], F32, kind="ExternalInput")
    bk_h = nc.dram_tensor("bk", [C], F32, kind="ExternalInput")
    bv_h = nc.dram_tensor("bv", [C], F32, kind="ExternalInput")
    bo_h = nc.dram_tensor("bo", [C], F32, kind="ExternalInput")
    bo_bf_h = nc.dram_tensor("bo_bf", [1, C], BF16, kind="ExternalInput")
    Eha_h = nc.dram_tensor("Eha", [1, P], F32, kind="ExternalInput")
    Ehb_h = nc.dram_tensor("Ehb", [1, P], F32, kind="ExternalInput")
    projT_h = nc.dram_tensor("projT", [P, MP], BF16, kind="ExternalInput")
    lng_h = nc.dram_tensor("ln_g", [C], F32, kind="ExternalInput")
    lnb_h = nc.dram_tensor("ln_b", [C], F32, kind="ExternalInput")
    out_h = nc.dram_tensor("out", [ROWS, C], F32, kind="ExternalOutput")
    if dbg:
        dbg_kT = nc.dram_tensor("dbg_kT", [P, 4, ROWS], BF16, kind="ExternalOutput")
        dbg_qT = nc.dram_tensor("dbg_qT", [P, 4, ROWS], BF16, kind="ExternalOutput")
        dbg_va = nc.dram_tensor("dbg_va", [P, ROWS // P, H, 65], BF16,
                                kind="ExternalOutput")
        dbg_ctx = nc.dram_tensor("dbg_ctx", [P, MC, B * H, 65], BF16,
                                 kind="ExternalOutput")
        dbg_outt = nc.dram_tensor("dbg_outt", [P, 4, 512], BF16,
                                  kind="ExternalOutput")

    h = h_h.ap()
    out = out_h.ap()

    def bcast_row(hnd):
        ap = hnd.ap()
        return bass.AP(tensor=ap.tensor, offset=ap.offset,
                       ap=[[0, P]] + list(ap.ap))

    with tile.TileContext(nc) as tc:
        import contextlib
        with contextlib.ExitStack() as ctx:
            consts = ctx.enter_context(tc.tile_pool(name="consts", bufs=1))
            persist = ctx.enter_context(tc.tile_pool(name="persist", bufs=1))

            # ---- replicated weights / constants -------------------------
            Wq_sb = consts.tile([P, 4, C], BF16)
            Wk_sb = consts.tile([P, 4, C], BF16)
            Wv_sb = consts.tile([P, 4, C], BF16)
            for ci in range(4):
                nc.sync.dma_start(Wq_sb[:, ci, :], Wq_h.ap()[ci * P:(ci + 1) * P, :])
                nc.sync.dma_start(Wk_sb[:, ci, :], Wk_h.ap()[ci * P:(ci + 1) * P, :])
                nc.sync.dma_start(Wv_sb[:, ci, :], Wv_h.ap()[ci * P:(ci + 1) * P, :])
            # Wo in head-pair layout: partitions 0:64 <- head 2k, 64:128 <- 2k+1
            Wo_sb = consts.tile([P, 4, C], BF16)
            for hp in range(4):
                nc.sync.dma_start(Wo_sb[0:D, hp, :],
                                  Wo_h.ap()[(2 * hp) * D:(2 * hp + 1) * D, :])
                nc.sync.dma_start(Wo_sb[D:P, hp, :],
                                  Wo_h.ap()[(2 * hp + 1) * D:(2 * hp + 2) * D, :])

            projT = consts.tile([P, MP], BF16)
            nc.sync.dma_start(projT, projT_h.ap())

            bq_sb = consts.tile([P, 4], F32)
            bk_sb = consts.tile([P, 4], F32)
            for ci in range(4):
                nc.sync.dma_start(bq_sb[:, ci:ci + 1], bq_h.ap()[ci * P:(ci + 1) * P])
                nc.sync.dma_start(bk_sb[:, ci:ci + 1], bk_h.ap()[ci * P:(ci + 1) * P])

            bv_b = consts.tile([P, C], F32)
            bo_b = consts.tile([P, C], F32)
            lng_b = consts.tile([P, C], F32)
            lnb_b = consts.tile([P, C], F32)
            nc.gpsimd.dma_start(out=bv_b, in_=bcast_row(bv_h))
            nc.gpsimd.dma_start(out=bo_b, in_=bcast_row(bo_h))
            nc.gpsimd.dma_start(out=lng_b, in_=bcast_row(lng_h))
            nc.gpsimd.dma_start(out=lnb_b, in_=bcast_row(lnb_h))

            lneps_sb = consts.tile([P, 1], F32)
            nc.vector.memset(lneps_sb, LNEPS)
            # Eha/Ehb broadcast a [1,512] row into partitions 0:64 / 64:128
            Eha = consts.tile([1, P], F32)
            Ehb = consts.tile([1, P], F32)
            nc.sync.dma_start(Eha, Eha_h.ap())
            nc.sync.dma_start(Ehb, Ehb_h.ap())
            ones_1 = consts.tile([1, P], BF16)
            nc.vector.memset(ones_1, 1.0)
            bo_bf = consts.tile([1, C], BF16)
            nc.sync.dma_start(bo_bf, bo_bf_h.ap())

            # ---- persistent activations --------------------------------
            kT_all = persist.tile([P, 4, ROWS], BF16)    # 32KB/part
            qT_all = persist.tile([P, 4, ROWS], BF16)    # 32KB/part
            vaug_all = persist.tile([P, ROWS // P, H, 65], BF16)  # 33.3KB/part
            nc.vector.memset(vaug_all[:, :, :, 64:65], 1.0)
            ctx_sb = persist.tile([P, MC, B * H, 65], BF16)  # 6.25KB/part
            ctx_f32 = persist.tile([P, MC, H, 65], F32, tag="ctx_f32")
            nc.vector.memset(ctx_f32, 0.0)

            # ---- hT via DMA transpose (released after stage A) ---------
            hT_pool = tc.alloc_tile_pool(name="hT", bufs=1)
            hT = hT_pool.tile([P, 4, ROWS], BF16)        # 32KB/part
            for ci in range(4):
                nc.sync.dma_start_transpose(hT[:, ci, :],
                                            h[:, ci * P:(ci + 1) * P])

            # ---- DRAM buffers for the two AllReduces -------------------
            dram = ctx.enter_context(tc.tile_pool(name="dram", bufs=1,
                                                  space="DRAM"))
            cc_in = []
            cc_out = []
            for b in range(B):
                cci = dram.tile([M, H, 65], F32, tag=f"ccin{b}")
                cco = dram.tile([M, H, 65], F32, addr_space="Shared",
                                tag=f"ccout{b}")
                cc_in.append(cci)
                cc_out.append(cco)

            # ================= stage A: qT/kT/v =========================
            with contextlib.ExitStack() as pa:
                ps_a = pa.enter_context(
                    tc.tile_pool(name="ps_a", bufs=4, space="PSUM"))
                for nb in range(NB):
                    n0 = nb * 512
                    for co in range(4):
                        pq = ps_a.tile([P, 512], F32, tag="ps_a")
                        for ci in range(4):
                            nc.tensor.matmul(pq, Wq_sb[:, ci, co * P:(co + 1) * P],
                                             hT[:, ci, n0:n0 + 512],
                                             start=(ci == 0), stop=(ci == 3))
                        nc.vector.tensor_scalar(qT_all[:, co, n0:n0 + 512], pq,
                                                bq_sb[:, co:co + 1], None, OP.add)
                        pk = ps_a.tile([P, 512], F32, tag="ps_a")
                        for ci in range(4):
                            nc.tensor.matmul(pk, Wk_sb[:, ci, co * P:(co + 1) * P],
                                             hT[:, ci, n0:n0 + 512],
                                             start=(ci == 0), stop=(ci == 3))
                        nc.vector.tensor_scalar(kT_all[:, co, n0:n0 + 512], pk,
                                                bk_sb[:, co:co + 1], None, OP.add)
                    for ns in range(4):
                        ch = nb * 4 + ns
                        pv = ps_a.tile([P, 512], F32, tag="ps_a")
                        for ci in range(4):
                            nc.tensor.matmul(pv, hT[:, ci, n0 + ns * P:n0 + (ns + 1) * P],
                                             Wv_sb[:, ci, :],
                                             start=(ci == 0), stop=(ci == 3))
                        # strided write into [ch, hd, 0:64] slots (+bias)
                        nc.vector.scalar_tensor_tensor(
                            vaug_all[:, ch, :, 0:64],
                            pv[:, :].rearrange("p (h d) -> p h d", h=H), 1.0,
                            bv_b[:, :].rearrange("p (h d) -> p h d", h=H),
                            OP.mult, OP.add)
            hT_pool.release()

            # ============ stage B + C: context, AllReduce ===============
            def stage_b(b):
                with contextlib.ExitStack() as pb:
                    kp_pool = pb.enter_context(tc.tile_pool(name="kp", bufs=4))
                    cstage = pb.enter_context(tc.tile_pool(name="cstage", bufs=4))
                    ps_kp = pb.enter_context(
                        tc.tile_pool(name="ps_kp", bufs=4, space="PSUM"))
                    ps_ctx = pb.enter_context(
                        tc.tile_pool(name="ps_ctx", bufs=4, space="PSUM"))
                    for hp in range(4):
                        pctx0 = ps_ctx.tile([P, MC * 65], F32, tag="ps_ctx")
                        pctx1 = ps_ctx.tile([P, MC * 65], F32, tag="ps_ctx")
                        for ch in range(NCH):
                            n1 = b * NSH + ch * P
                            gch = b * NCH + ch
                            pkp0 = ps_kp.tile([P, MP], F32, tag="ps_kp")
                            pkp1 = ps_kp.tile([P, MP], F32, tag="ps_kp")
                            nc.tensor.matmul(pkp0, kT_all[0:D, hp, n1:n1 + P],
                                             projT[0:D, :], start=True, stop=True)
                            nc.tensor.matmul(pkp1, kT_all[D:P, hp, n1:n1 + P],
                                             projT[D:P, :], start=True, stop=True)
                            kp0 = kp_pool.tile([P, MP], BF16, tag="kp")
                            kp1 = kp_pool.tile([P, MP], BF16, tag="kp")
                            nc.vector.tensor_scalar(kp0, pkp0, 0.0, KEPS,
                                                    OP.max, OP.add)
                            nc.vector.tensor_scalar(kp1, pkp1, 0.0, KEPS,
                                                    OP.max, OP.add)
                            for mc in range(MC):
                                st = (ch == 0 and mc == 0)
                                sp = (ch == NCH - 1 and mc == MC - 1)
                                nc.tensor.matmul(
                                    pctx0[:, mc * 65:(mc + 1) * 65],
                                    kp0[:, mc * P:(mc + 1) * P],
                                    vaug_all[:, gch, 2 * hp, :],
                                    start=st, stop=sp)
                                nc.tensor.matmul(
                                    pctx1[:, mc * 65:(mc + 1) * 65],
                                    kp1[:, mc * P:(mc + 1) * P],
                                    vaug_all[:, gch, 2 * hp + 1, :],
                                    start=st, stop=sp)
                        # PSUM -> SBUF stage -> collective input buffer
                        cst0 = cstage.tile([P, MC * 65], F32, tag="cstage")
                        cst1 = cstage.tile([P, MC * 65], F32, tag="cstage")
                        nc.vector.tensor_copy(cst0, pctx0)
                        nc.vector.tensor_copy(cst1, pctx1)
                        for mc in range(MC):
                            w = MCW[mc]
                            nc.sync.dma_start(
                                cc_in[b][mc * P:mc * P + w, 2 * hp, :],
                                cst0[0:w, mc * 65:(mc + 1) * 65])
                            nc.sync.dma_start(
                                cc_in[b][mc * P:mc * P + w, 2 * hp + 1, :],
                                cst1[0:w, mc * 65:(mc + 1) * 65])

            def launch_ar(b):
                nc.gpsimd.collective_compute(
                    "AllReduce", OP.add,
                    replica_groups=[list(range(ncores))],
                    ins=[cc_in[b].opt()], outs=[cc_out[b].opt()],
                )

            def fetch_ctx(b):
                for mc in range(MC):
                    w = MCW[mc]
                    nc.sync.dma_start(
                        ctx_f32[0:w, mc, :, :],
                        cc_out[b][mc * P:mc * P + w, :, :])
                nc.vector.tensor_copy(ctx_sb[:, :, b * H:(b + 1) * H, :],
                                      ctx_f32)

            stage_b(0)
            launch_ar(0)
            stage_b(1)
            launch_ar(1)

            # ================= stage D: output pass =====================
            with contextlib.ExitStack() as pd:
                qp_pool = pd.enter_context(tc.tile_pool(name="qp", bufs=2))
                outt_pool = pd.enter_context(tc.tile_pool(name="outt", bufs=2))
                den_pool = pd.enter_context(tc.tile_pool(name="den", bufs=4))
                y_pool = pd.enter_context(tc.tile_pool(name="y", bufs=3))
                st_pool = pd.enter_context(tc.tile_pool(name="st", bufs=4))
                ps_qp = pd.enter_context(
                    tc.tile_pool(name="ps_qp", bufs=2, space="PSUM"))
                ps_out = pd.enter_context(
                    tc.tile_pool(name="ps_out", bufs=3, space="PSUM"))
                ps_misc = pd.enter_context(
                    tc.tile_pool(name="ps_misc", bufs=3, space="PSUM"))

                for nb in range(NB):
                    b = nb // 4
                    if nb % 4 == 0:
                        fetch_ctx(b)
                    n0 = nb * 512
                    OUTT = outt_pool.tile([P, 4, 512], BF16, tag="outt")
                    for hp in range(4):
                        pair0 = b * H + 2 * hp
                        qp0 = qp_pool.tile([P, MC, 512], BF16, tag="qp")
                        qp1 = qp_pool.tile([P, MC, 512], BF16, tag="qp")
                        for mc in range(MC):
                            pqp0 = ps_qp.tile([P, 512], F32, tag="ps_qp")
                            pqp1 = ps_qp.tile([P, 512], F32, tag="ps_qp")
                            nc.tensor.matmul(pqp0, projT[0:D, mc * P:(mc + 1) * P],
                                             qT_all[0:D, hp, n0:n0 + 512],
                                             start=True, stop=True)
                            nc.tensor.matmul(pqp1, projT[D:P, mc * P:(mc + 1) * P],
                                             qT_all[D:P, hp, n0:n0 + 512],
                                             start=True, stop=True)
                            nc.vector.tensor_scalar(qp0[:, mc, :], pqp0, 0.0, KEPS,
                                                    OP.max, OP.add)
                            nc.vector.tensor_scalar(qp1[:, mc, :], pqp1, 0.0, KEPS,
                                                    OP.max, OP.add)
                        pout0 = ps_out.tile([65, 512], F32, tag="ps_out")
                        pout1 = ps_out.tile([65, 512], F32, tag="ps_out")
                        for mc in range(MC):
                            w = MCW[mc]
                            nc.tensor.matmul(pout0, ctx_sb[0:w, mc, pair0, :],
                                             qp0[0:w, mc, :],
                                             start=(mc == 0), stop=(mc == 2))
                            nc.tensor.matmul(pout1, ctx_sb[0:w, mc, pair0 + 1, :],
                                             qp1[0:w, mc, :],
                                             start=(mc == 0), stop=(mc == 2))
                        dinva = den_pool.tile([1, 512], F32, tag="dinva")
                        dinvb = den_pool.tile([1, 512], F32, tag="dinvb")
                        nc.vector.reciprocal(dinva, pout0[64:65, :])
                        nc.vector.reciprocal(dinvb, pout1[64:65, :])
                        pb2 = ps_misc.tile([P, 512], F32, tag="ps_misc")
                        nc.tensor.matmul(pb2, Eha[:, :], dinva[:, :],
                                         start=True, stop=False)
                        nc.tensor.matmul(pb2, Ehb[:, :], dinvb[:, :],
                                         start=False, stop=True)
                        pbs = den_pool.tile([P, 512], F32, tag="pbs")
                        nc.vector.tensor_copy(pbs, pb2)
                        nc.vector.tensor_tensor(OUTT[0:D, hp, :], pout0[0:D, :],
                                                pbs[0:D, :], OP.mult)
                        nc.vector.tensor_tensor(OUTT[D:P, hp, :], pout1[0:D, :],
                                                pbs[D:P, :], OP.mult)

                    for ns in range(4):
                        py = ps_misc.tile([P, 512], F32, tag="ps_misc")
                        for hp in range(4):
                            nc.tensor.matmul(py, OUTT[:, hp, ns * P:(ns + 1) * P],
                                             Wo_sb[:, hp, :],
                                             start=(hp == 0), stop=False)
                        nc.tensor.matmul(py, ones_1, bo_bf,
                                         start=False, stop=True)
                        stats = st_pool.tile([P, 6], F32, tag="stats")
                        nc.vector.bn_stats(stats, py)
                        mv = st_pool.tile([P, 2], F32, tag="mv")
                        nc.vector.bn_aggr(mv, stats)
                        std = st_pool.tile([P, 1], F32, tag="std")
                        nc.scalar.activation(std, mv[:, 1:2], AF.Sqrt,
                                             bias=lneps_sb[:])
                        rstd = st_pool.tile([P, 1], F32, tag="rstd")
                        nc.vector.reciprocal(rstd, std)
                        nmr = st_pool.tile([P, 1], F32, tag="nmr")
                        nc.vector.tensor_scalar(nmr, mv[:, 0:1], -1.0, rstd,
                                                OP.mult, OP.mult)
                        ynorm = y_pool.tile([P, C], F32, tag="ynorm")
                        # Identity accepts per-partition AP scale/bias (Copy
                        # does not) and shares every ACT table set (no
                        # reloads when mixed with Sqrt).
                        nc.scalar.activation(ynorm, py, AF.Identity,
                                             bias=nmr[:], scale=rstd[:])
                        yg = y_pool.tile([P, C], F32, tag="yg")
                        nc.vector.tensor_tensor(yg, ynorm, lng_b, OP.mult)
                        yfin = y_pool.tile([P, C], F32, tag="yfin")
                        nc.vector.tensor_tensor(yfin, yg, lnb_b, OP.add)
                        nc.sync.dma_start(out[n0 + ns * P:n0 + (ns + 1) * P, :],
                                          yfin)
                    if dbg and nb == 0:
                        nc.sync.dma_start(dbg_outt.ap(), OUTT[:, :, :])

                if dbg:
                    nc.sync.dma_start(dbg_kT.ap(), kT_all[:, :, :])
                    nc.sync.dma_start(dbg_qT.ap(), qT_all[:, :, :])
                    nc.sync.dma_start(dbg_va.ap(), vaug_all[:, :, :, :])
                    nc.sync.dma_start(dbg_ctx.ap(), ctx_sb[:, :, :, :])

    return nc


_NC = None
LAST_RESULT = None


def make_in_maps(inputs):
    bf = mybir.dt.np(BF16)
    h = np.asarray(inputs["h"], dtype=np.float32)
    proj = np.asarray(inputs["proj"], dtype=np.float32)     # [M, D]
    projT = np.zeros((P, MP), dtype=np.float32)
    projT[0:D, 0:M] = proj.T
    projT[D:P, 0:M] = proj.T
    shared = {
        "Wq": np.asarray(inputs["Wq"], dtype=np.float32).astype(bf),
        "Wk": np.asarray(inputs["Wk"], dtype=np.float32).astype(bf),
        "Wv": np.asarray(inputs["Wv"], dtype=np.float32).astype(bf),
        "Wo": np.asarray(inputs["Wo"], dtype=np.float32).astype(bf),
        "projT": projT.astype(bf),
        "bq": np.ascontiguousarray(inputs["bq"], dtype=np.float32),
        "bk": np.ascontiguousarray(inputs["bk"], dtype=np.float32),
        "bv": np.ascontiguousarray(inputs["bv"], dtype=np.float32),
        "bo": np.ascontiguousarray(inputs["bo"], dtype=np.float32),
        "bo_bf": np.asarray(inputs["bo"], dtype=np.float32).reshape(1, C).astype(bf),
        "Eha": np.concatenate([np.ones((1, D), np.float32),
                               np.zeros((1, D), np.float32)], axis=1),
        "Ehb": np.concatenate([np.zeros((1, D), np.float32),
                               np.ones((1, D), np.float32)], axis=1),
        "ln_g": np.ascontiguousarray(inputs["ln_g"], dtype=np.float32),
        "ln_b": np.ascontiguousarray(inputs["ln_b"], dtype=np.float32),
    }
    in_maps = []
    for c in range(NCORES):
        shard = np.ascontiguousarray(
            h[:, c * NSH:(c + 1) * NSH, :].reshape(ROWS, C)).astype(bf)
        m = {"h": shard}
        m.update(shared)
        in_maps.append(m)
    return in_maps


def get_nc():
    global _NC
    if _NC is None:
        _NC = build_nc()
        _NC.finalize()
    return _NC


def kernel(**inputs):
    global LAST_RESULT
    nc = get_nc()
    from concourse.bass_utils import run_bass_kernel_spmd

    in_maps = make_in_maps(inputs)
    res = run_bass_kernel_spmd(nc, in_maps, core_ids=list(range(NCORES)))
    LAST_RESULT = res
    out = np.empty((B, N, C), dtype=np.float32)
    for c in range(NCORES):
        out[:, c * NSH:(c + 1) * NSH, :] = res.results[c]["out"].reshape(B, NSH, C)
    return out
